# revision 13
# baseline (speedup 1.0000x reference)
"""Multi-head attention (B=4, S=2048, D=768, H=12) on 8 Trainium2 cores.

Sharding: 48 (batch, head) pairs split 6-per-core; core c handles batch
c//2, local heads 6*(c%2) .. 6*(c%2)+5.  Each core computes its heads'
probs [6, S, S] (the 100MB/core memory-roofline term) and its ctx
columns [S, 384]; the host reassembles the full outputs.

Pipeline per core (bf16 matmuls, fp32 accumulation):
  hs -> hsT (PE transpose)  ->  Q^T/K^T/V^T head-pair projections
  scores = (qT|1)^T @ (kT|maskrow)  (K=65 matmul folds the mask add)
  exp on ScalarE (scale=1/8, accum_out row sums, no row-max: scores ~N(0,1))
  probs_f32 = exp * (1/sum)  on VectorE -> DMA to HBM
  expT via PE transposes -> ctx^T = sum_k v_chunk^T.T @ expT  -> scale, DMA.
"""

import numpy as np
import ml_dtypes

import concourse.bacc as bacc
import concourse.bass as bass
import concourse.tile as tile
from concourse import mybir
from concourse.masks import make_identity

B, S, D = 4, 2048, 768
H, HD = 12, 64
NCORES = 8
HPC = H * B // NCORES  # 6 heads per core
SCALE = 1.0 / np.sqrt(D)  # weight prescale used by reference setup (unused here)
INV_SQRT_HD = 0.125

F32 = mybir.dt.float32
BF16 = mybir.dt.bfloat16

NQ = S // 128  # 16 query chunks
NC_D = D // 128  # 6 contraction chunks for projections
NK = S // 128  # 16 key chunks


def build_bass():
    nc = bacc.Bacc()
    hs = nc.declare_dram_parameter("hs", [S, D], BF16, isOutput=False)
    w3 = nc.declare_dram_parameter("w3", [D, 3 * HPC * HD], BF16, isOutput=False)
    b3 = nc.declare_dram_parameter("b3", [3 * HPC * HD], F32, isOutput=False)
    maskrow = nc.declare_dram_parameter("maskrow", [S], BF16, isOutput=False)
    probs_out = nc.declare_dram_parameter("probs_out", [HPC, S, S], F32, isOutput=True)
    ctx_out = nc.declare_dram_parameter("ctx_out", [S, HPC * HD], F32, isOutput=True)

    with tile.TileContext(nc) as tc:
        with (
            tc.tile_pool(name="consts", bufs=1) as consts,
            tc.tile_pool(name="psA", bufs=2, space="PSUM") as psA,
            tc.tile_pool(name="psB", bufs=2, space="PSUM") as psB,
            tc.tile_pool(name="psC", bufs=2, space="PSUM") as psC,
            tc.tile_pool(name="qk", bufs=1) as qk,
            tc.tile_pool(name="expp", bufs=3) as expp,
            tc.tile_pool(name="prbp", bufs=2) as prbp,
            tc.tile_pool(name="ptp", bufs=2) as ptp,
            tc.tile_pool(name="stat", bufs=4) as stat,
            tc.tile_pool(name="recp", bufs=10) as recp,
            tc.tile_pool(name="ctxp", bufs=3) as ctxp,
        ):
            ident_bf = consts.tile([128, 128], BF16)
            make_identity(nc, ident_bf)
            ident_f32 = consts.tile([128, 128], F32)
            make_identity(nc, ident_f32)

            # ---- constants: weights, biases, mask ----
            wb = consts.tile([128, NC_D, 3 * HPC * HD], BF16)  # [p, c, n]
            w3r = w3.rearrange("(c p) n -> p c n", p=128)
            for t in range(3):
                nc.sync.dma_start(
                    out=wb[:, :, t * 3 * HD * 2 : (t + 1) * 3 * HD * 2],
                    in_=w3r[:, :, t * 3 * HD * 2 : (t + 1) * 3 * HD * 2],
                )
            # bias columns: j = t*3 + pp  -> b3[(t*3+pp)*128 : +128]
            bias_sb = consts.tile([128, 9], F32)
            nc.sync.dma_start(out=bias_sb, in_=b3.rearrange("(j p) -> p j", p=128))
            maskc = consts.tile([1, S], BF16)
            nc.sync.dma_start(out=maskc, in_=maskrow[None, :])

            # ---- hs (bf16, natural) then hs -> hsT via PE transposes ----
            hsb = consts.tile([128, NQ, D], BF16)  # hsb[p, si, d] = hs[si*128+p, d]
            hsr = hs.rearrange("(si p) d -> p si d", p=128)
            for si in range(NQ):
                nc.sync.dma_start(out=hsb[:, si, :], in_=hsr[:, si, :])
            hsT = consts.tile([128, NC_D, S], BF16)  # [p, c, s] = hs[s, c*128+p]
            for si in range(NQ):
                for c in range(NC_D):
                    pt = psB.tile([128, 128], BF16, tag="mix")
                    nc.tensor.transpose(
                        pt, hsb[:, si, c * 128 : (c + 1) * 128], ident_bf
                    )
                    nc.any.tensor_copy(
                        out=hsT[:, c, si * 128 : (si + 1) * 128], in_=pt
                    )

            # ---- per head-pair ----
            for pp in range(3):
                qTs = [
                    qk.tile([65, S], BF16, tag=f"qT{i}", name=f"qT{i}") for i in range(2)
                ]
                kTs = [
                    qk.tile([65, S], BF16, tag=f"kT{i}", name=f"kT{i}") for i in range(2)
                ]
                vT2 = qk.tile([128, S], BF16, tag="vT2")
                v2 = qk.tile([128, NK, 128], BF16, tag="v2")  # [s_in_chunk, c, dpair]

                for t in range(3):  # q, k, v
                    j = t * 3 + pp
                    for sb_i in range(4):  # s blocks of 512
                        ps = psB.tile([128, 512], F32, tag="mix")
                        for c in range(NC_D):
                            nc.tensor.matmul(
                                ps,
                                wb[:, c, j * 128 : (j + 1) * 128],
                                hsT[:, c, sb_i * 512 : (sb_i + 1) * 512],
                                start=(c == 0),
                                stop=(c == NC_D - 1),
                            )
                        sl = slice(sb_i * 512, (sb_i + 1) * 512)
                        if t == 0:
                            nc.vector.tensor_scalar_add(
                                qTs[0][0:64, sl], ps[0:64, :], bias_sb[0:64, j : j + 1]
                            )
                            nc.vector.tensor_scalar_add(
                                qTs[1][0:64, sl], ps[64:128, :], bias_sb[64:128, j : j + 1]
                            )
                        elif t == 1:
                            nc.vector.tensor_scalar_add(
                                kTs[0][0:64, sl], ps[0:64, :], bias_sb[0:64, j : j + 1]
                            )
                            nc.vector.tensor_scalar_add(
                                kTs[1][0:64, sl], ps[64:128, :], bias_sb[64:128, j : j + 1]
                            )
                        else:
                            nc.vector.tensor_scalar_add(
                                vT2[:, sl], ps, bias_sb[:, j : j + 1]
                            )

                # v natural layout: v2[:, c, :] = v rows c*128..c*128+127 (128 = 2 heads x 64)
                for c in range(NK):
                    pt = psB.tile([128, 128], BF16, tag="mix")
                    nc.tensor.transpose(pt, vT2[:, c * 128 : (c + 1) * 128], ident_bf)
                    nc.any.tensor_copy(out=v2[:, c, :], in_=pt)

                for hh in range(2):
                    h_local = pp * 2 + hh
                    qT, kT = qTs[hh], kTs[hh]
                    nc.vector.memset(qT[64:65, :], 1.0)
                    nc.vector.tensor_copy(out=kT[64:65, :], in_=maskc)

                    recs = {}
                    probsT = None
                    for qi in range(NQ):
                        jq = qi % 4
                        if jq == 0:
                            probsT = ptp.tile([128, NK, 4, 128], BF16, tag="probsT")
                        acc = stat.tile([128, 2], F32, tag="acc")
                        expq = expp.tile([128, S], BF16, tag="expq")
                        for half in range(2):
                            ps = psA.tile([128, 1024], F32, tag="sc")
                            for nb in range(2):
                                nc.tensor.matmul(
                                    ps[:, nb * 512 : (nb + 1) * 512],
                                    qT[:, qi * 128 : (qi + 1) * 128],
                                    kT[:, half * 1024 + nb * 512 : half * 1024 + (nb + 1) * 512],
                                    start=True,
                                    stop=True,
                                )
                            nc.scalar.activation(
                                out=expq[:, half * 1024 : (half + 1) * 1024],
                                in_=ps,
                                func=mybir.ActivationFunctionType.Exp,
                                scale=INV_SQRT_HD,
                                accum_out=acc[:, half : half + 1],
                            )
                        sumt = stat.tile([128, 1], F32, tag="sumt")
                        nc.vector.tensor_add(sumt, acc[:, 0:1], acc[:, 1:2])
                        recip = recp.tile([128, 1], F32, tag="recip")
                        nc.vector.reciprocal(recip, sumt)
                        recs[qi] = recip

                        prb = prbp.tile([128, S], F32, tag="prb")
                        nc.vector.tensor_scalar_mul(prb, expq, recip)
                        nc.sync.dma_start(
                            out=probs_out[h_local, qi * 128 : (qi + 1) * 128, :],
                            in_=prb,
                        )

                        for g in range(4):
                            pk = psB.tile([128, 4, 128], BF16, tag="mix")
                            for cc in range(4):
                                c = 4 * g + cc
                                nc.tensor.transpose(
                                    pk[:, cc, :],
                                    expq[:, c * 128 : (c + 1) * 128],
                                    ident_bf,
                                )
                            nc.any.tensor_copy(
                                out=probsT[:, 4 * g : 4 * g + 4, jq, :], in_=pk
                            )

                        if jq == 3:
                            qb = qi // 4
                            cps = psC.tile([64, 512], F32, tag="ctx")
                            for c in range(NK):
                                nc.tensor.matmul(
                                    cps,
                                    v2[:, c, hh * 64 : (hh + 1) * 64],
                                    probsT[:, c, :, :],
                                    start=(c == 0),
                                    stop=(c == NK - 1),
                                )
                            csb = ctxp.tile([64, 512], F32, tag="csb")
                            nc.any.tensor_copy(out=csb, in_=cps)
                            for jj in range(4):
                                tps = psB.tile([128, 64], F32, tag="mix")
                                nc.tensor.transpose(
                                    tps,
                                    csb[:, jj * 128 : (jj + 1) * 128],
                                    ident_f32[0:64, 0:64],
                                )
                                cf = ctxp.tile([128, 64], F32, tag="cf")
                                nc.vector.tensor_scalar_mul(cf, tps, recs[qb * 4 + jj])
                                r = qb * 4 + jj
                                nc.sync.dma_start(
                                    out=ctx_out[
                                        r * 128 : (r + 1) * 128,
                                        h_local * 64 : (h_local + 1) * 64,
                                    ],
                                    in_=cf,
                                )
    nc.finalize()
    return nc


_NC_CACHE = None


def _get_nc():
    global _NC_CACHE
    if _NC_CACHE is None:
        _NC_CACHE = build_bass()
    return _NC_CACHE


def make_in_maps(hidden_states, attention_mask, Wq, bq, Wk, bk, Wv, bv):
    hidden_states = np.asarray(hidden_states, dtype=np.float32)
    attention_mask = np.asarray(attention_mask, dtype=np.float32)
    Wq, Wk, Wv = (np.asarray(w, dtype=np.float32) for w in (Wq, Wk, Wv))
    bq, bk, bv = (np.asarray(b, dtype=np.float32) for b in (bq, bk, bv))

    in_maps = []
    for core in range(NCORES):
        b = core // 2
        h0 = HPC * (core % 2)
        cols = slice(h0 * HD, (h0 + HPC) * HD)
        w3 = np.ascontiguousarray(
            np.concatenate([Wq[:, cols], Wk[:, cols], Wv[:, cols]], axis=1)
        ).astype(ml_dtypes.bfloat16)
        b3 = np.concatenate([bq[cols], bk[cols], bv[cols]])
        # scores get multiplied by 1/8 inside exp; pre-scale mask add by 8 so
        # the net additive term matches the reference's -10000*(1-mask).
        maskrow = ((1.0 - attention_mask[b]) * (-80000.0)).astype(ml_dtypes.bfloat16)
        in_maps.append(
            {
                "hs": np.ascontiguousarray(hidden_states[b]).astype(ml_dtypes.bfloat16),
                "w3": w3,
                "b3": np.ascontiguousarray(b3),
                "maskrow": maskrow,
            }
        )
    return in_maps


def run(in_maps, **kwargs):
    from concourse.bass_utils import run_bass_kernel_spmd

    nc = _get_nc()
    return run_bass_kernel_spmd(nc, in_maps, core_ids=list(range(NCORES)), **kwargs)


def kernel(hidden_states, attention_mask, Wq, bq, Wk, bk, Wv, bv):
    in_maps = make_in_maps(hidden_states, attention_mask, Wq, bq, Wk, bk, Wv, bv)
    res = run(in_maps)
    ctx = np.empty((B, S, D), dtype=np.float32)
    probs = np.empty((B, H, S, S), dtype=np.float32)
    for core in range(NCORES):
        b = core // 2
        h0 = HPC * (core % 2)
        r = res.results[core]
        probs[b, h0 : h0 + HPC] = r["probs_out"]
        ctx[b, :, h0 * HD : (h0 + HPC) * HD] = r["ctx_out"]
    return ctx, probs


if __name__ == "__main__":
    nc = build_bass()
    print("built ok")
